# revision 29
# baseline (speedup 1.0000x reference)
"""Trainium2 Bass kernel: pairwise L2 distance + softmax classifier head.

reference math (per row n of context, all m result rows):
    sq[n, m] = ||c_n||^2 - 2 c_n.r_m + ||r_m||^2
    out[n, m] = 1 - softmax_m(sqrt(sq[n, m]))

Strategy
- Data-parallel over the context axis: 8 cores x 128 context rows each;
  result_embeddings replicated to every core.
- Host pre-transposes both operands into contraction-major ("[d, .]")
  chunk layouts so the tensor engine needs no on-chip transposes.
- Per core: 8 accumulating matmuls give cross = C_i @ R^T in PSUM
  (interleaved in half-K groups with the ones-weight square-norm
  reductions so the epilogue-gating matmuls retire early); K=1 matmuls
  broadcast -0.5*(r_sq[m] + c_sq[n]) into the same PSUM (rows
  mean-centered by -D so bf16 rounds only small residuals); the scalar
  engine computes dist = exp(0.5*ln(-2P)) and e = exp(dist) with fused
  row-sum accumulation, and DVE finishes with 1 - e/s.  sqrt-as-
  exp(ln/2) keeps every activation in ONE table set
  (natural_log_exp_and_others, forced via the insert_act_table_loads
  patch) - each avoided table switch is 1.28us.  No softmax
  max-subtraction is needed: distances <= ~51, far below f32 overflow.
- Built on bacc.Bacc + compile(): the ISA Events field fits exactly one
  semaphore wait + one update per instruction; bacc's passes lower the
  Tile-generated multi-wait sync into encodable event-semaphore chains.

Steady-state tuning (measured via in-NEFF repeat slope on HW, which
cancels the ~3.7ms/dispatch axon PJRT overhead and the ~69ms tunnel
round-trip; see test.py):
- BK_SQ_FUSE=4 (default): square 4 contraction chunks per DVE
  instruction instead of 1 - each DVE instruction carries ~105ns of
  fixed overhead (58-cycle SBUF access + 45ns SEQ), so 16 small squares
  waste ~1.4us/iteration; groups of 4 keep enough pipelining against
  the PE norm-matmul consumers while cutting 12 of those overheads.
- BK_ACT_SQ=1 (default): the last rT square group runs on the ACT
  engine (Square is in the natural_log_exp_and_others table set - no
  table switch), rebalancing the DVE-critical square chain.
- BK_OUT_BF16=1 (default): bf16 output DMA (64KB not 128KB per core);
  output values are ~1 so bf16 keeps rel err ~2.4e-3, far inside the
  2e-2 gate. kernel() casts back to float32 on host.
HW steady-state: ~2.6us/iteration per core (from 3.5us before these
three changes; TimelineSim models 2.9us). Engines are near-balanced:
DVE ~2.3us, PE ~2.4us, ACT ~2.3us busy per iteration.
Rejected by measurement/sim: fp8 inputs (no PE-rate or total-time win
in the cost model, DMA is not the binding resource), fully-fused
single-op squares (serializes the norm-matmul tail), final combine on
ACT (ACT is critical in the epilogue chain), pre-reducing square
chunks on DVE (DVE is critical), 1 or 4 rT DMAs (2 is best).
"""

import os

import ml_dtypes
import numpy as np

import concourse.bass as bass
import concourse.mybir as mybir
import concourse.tile as tile
from concourse import bacc
from concourse.bass_utils import run_bass_kernel_spmd

N, D, M = 1024, 1024, 256
NCORES = 8
NPC = N // NCORES  # context rows per core (output partition dim)
KC = D // 128  # contraction chunks of 128

# "bf16": inputs cast to bf16 on host (half the DMA bytes, ~5e-4 rel err)
# "fp8" : inputs fp8 e4m3; cross+norm matmuls in MatmulPerfMode.DoubleRow
#         (0.5 PE cycles/row, K=256/instruction, halves PE time and input
#         DMA; ~1e-2 rel err). REJECTED: fp8 is a 1-byte dtype, so the
#         DVE/ACT elementwise 2x fast path (2-byte only) is lost and the
#         square ops double in cost - sim: 3046ns/body (best placement,
#         ctx squares on gpsimd) vs 2912 for bf16. ALSO fails walrus
#         lowering in this env (CallFunctionObjArgs error in lower_act/
#         lower_dve). Kept for reference only.
# "f32r": f32 inputs, matmuls in float32r (full-rate PE, ~tf32 products)
# "f32" : f32 inputs, plain fp32 matmuls (1/4-rate PE, ~2e-6 rel err)
MODE = os.environ.get("BASS_KERNEL_MODE", "bf16")


def build_nc(mode: str = MODE) -> bass.Bass:
    AF = mybir.ActivationFunctionType
    f32 = mybir.dt.float32
    dt_in = {
        "fp8": mybir.dt.float8e4,
        "bf16": mybir.dt.bfloat16,
        "f32r": mybir.dt.float32r,
        "f32": f32,
    }[mode]
    DR = mode == "fp8"  # DoubleRow: pair adjacent K-chunks per matmul
    DRM = mybir.MatmulPerfMode.DoubleRow

    def mm(ap):
        return ap

    # bacc (not raw Bass): its compile() pass lowers multi-wait sync_info
    # into event-semaphore sequences the ISA can actually encode (the
    # Events field fits exactly one wait + one update per instruction).
    nc = bacc.Bacc("TRN2", target_bir_lowering=False, debug=False, num_devices=NCORES)

    # The act-table chooser is first-match greedy; hide the pure-Ln set so
    # the Ln lands in natural_log_exp_and_others and the epilogue's Ln/Exp
    # chain needs exactly ONE activation-table load (each switch is 1.28us).
    def _act_loads(self=nc):
        import bass_rust as _bass_rust
        from concourse.hw_specs import get_activation_tables
        has_act = any(
            isinstance(i, mybir.InstActivation)
            for b in self.main_func.blocks
            for i in b.instructions
        )
        if not has_act:
            return
        tables = [
            (name, s if name == "natural_log_exp_and_others" else set())
            for name, s in get_activation_tables(self.m.arch).items()
        ]
        _bass_rust.insert_act_table_loads(self, tables)

    nc.insert_act_table_loads = _act_loads
    # ctxT[p, k*NPC + n] = C[core*NPC + n, k*128 + p]
    ctxT = nc.dram_tensor("ctxT", [128, KC * NPC], dt_in, kind="ExternalInput")
    # rT[p, k*M + m] = R[m, k*128 + p]
    rT = nc.dram_tensor("rT", [128, KC * M], dt_in, kind="ExternalInput")
    HOST_RSQ = os.environ.get("BK_HOST_RSQ", "1") == "1"
    HOST_CSQ = os.environ.get("BK_HOST_CSQ", "1") == "1"
    if HOST_CSQ:
        # ||c_n||^2 - D for this core's query rows, precomputed on host in
        # f32 (more accurate than the device bf16 square path) - [1, NPC]
        csqrow = nc.dram_tensor(
            "csqrow", [1, NPC], mybir.dt.bfloat16, kind="ExternalInput"
        )
    if HOST_RSQ:
        # ||r_m||^2 - D, precomputed with the (fixed) result-embedding
        # database on host - standard retrieval practice - as a bf16 row
        rsqrow = nc.dram_tensor(
            "rsqrow", [1, M], mybir.dt.bfloat16, kind="ExternalInput"
        )
    dt_out = mybir.dt.bfloat16 if os.environ.get("BK_OUT_BF16", "1") == "1" else f32
    out = nc.dram_tensor("out", [NPC, M], dt_out, kind="ExternalOutput")

    with tile.TileContext(nc) as tc:
        with (
            tc.tile_pool(name="cin", bufs=KC // 2) as cin,
            tc.tile_pool(name="rin", bufs=KC) as rin,
            tc.tile_pool(name="sq", bufs=KC) as sqp,
            tc.tile_pool(name="consts", bufs=1) as consts,
            tc.tile_pool(
                name="epi", bufs=int(os.environ.get("BK_EPI_BUFS", "1"))
            ) as epi,
            tc.tile_pool(
                name="psum", bufs=int(os.environ.get("BK_PSUM_BUFS", "1")),
                space="PSUM",
            ) as psp,
        ):
            row_dt = mybir.dt.bfloat16 if mode in ("bf16", "fp8") else f32
            ones_col = consts.tile([128, 1], dt_in)
            nc.vector.memset(ones_col, 1.0)
            if DR:
                # [128, 2, 1] ones: DoubleRow stationary operand for the
                # ones-weight norm reductions (2 K-chunks per matmul)
                ones2 = consts.tile([128, 2, 1], dt_in)
                nc.vector.memset(ones2, 1.0)
            neghalf = consts.tile([1, NPC], row_dt)
            nc.vector.memset(neghalf, -0.5)
            negh_row = consts.tile([1, M], row_dt)
            nc.vector.memset(negh_row, -0.5)
            mmean_row = consts.tile([1, NPC], row_dt)
            nc.vector.memset(mmean_row, -float(D))
            ones_row = consts.tile([1, M], row_dt)
            nc.vector.memset(ones_row, 1.0)
            if os.environ.get("BK_LN_BIAS", "0") == "1":
                # per-partition +2D bias column for the Ln (large scalar
                # biases need a const AP, immediates won't lower)
                lnb_col = consts.tile([128, 1], f32)
                nc.vector.memset(lnb_col, 2.0 * float(D))

            # HAM warmup: the PE clock-gate defaults to 1.2GHz and needs
            # ~3.4us of sustained activity to unthrottle to 2.4GHz.  The real
            # matmuls only start after the ~4us DMA wait, so without this
            # they all run cold.  Burn dummy matmuls on a const tile during
            # the DMA wait; results are discarded.
            nwarm = int(os.environ.get("BK_WARM_MM", "16"))
            if nwarm:
                warm_in = consts.tile([128, NPC], dt_in)
                nc.vector.memset(warm_in, 1.0)
                warm_ps = psp.tile([1, NPC], f32, tag="warm")
                for _ in range(nwarm):
                    nc.tensor.matmul(
                        warm_ps[:], ones_col[:], warm_in[:], start=True, stop=True
                    )

            if HOST_RSQ:
                # loop-invariant 512B row: own HWDGE queue, overlaps the big
                # input DMAs in the real kernel (adds ~0 to its makespan)
                rsq_dma = consts.tile([1, M], row_dt)
                nc.sync.dma_start(out=rsq_dma[:], in_=rsqrow[:, :])
            if HOST_CSQ:
                csq_dma = consts.tile([1, NPC], row_dt)
                nc.sync.dma_start(out=csq_dma[:], in_=csqrow[:, :])

            # BK_REPEAT replicates the body inside one NEFF (benchmarking:
            # separates per-iteration throughput from fixed overhead)
            for _rep in range(int(os.environ.get("BK_REPEAT", "1"))):
                P = psp.tile([NPC, M], f32, tag="P")
                rsq_ps = psp.tile([1, M], f32, tag="rsq")
                csq_ps = psp.tile([1, NPC], f32, tag="csq")

                # <=8 DMAs total so each rides its own HWDGE completion lane (a
                # reused lane forces a serializing queue wait on the out-DMA,
                # blowing the 1-wait-per-instruction sync budget).  Per-DMA fixed
                # costs (HWDGE issue + completion sem) are ~1.5us, so keep the
                # count low.
                CPD = KC // int(os.environ.get("BK_CTX_DMAS", "1"))
                RPD = KC // int(os.environ.get("BK_RT_DMAS", "2"))
                ctiles = [None] * (KC // CPD)
                rtiles = [None] * (KC // RPD)
                # interleave issue order (ctx0, rT0, ctx1, rT1, ...) so the
                # first cross-matmul's inputs finish the serialized transfer
                # stream as early as possible
                for j in range(max(len(ctiles), len(rtiles))):
                    if j < len(ctiles):
                        # 3D [128, CPD, NPC]: chunk index explicit so DR mode
                        # can slice adjacent-chunk pairs as [128, 2, NPC] APs
                        ct = cin.tile([128, CPD, NPC], dt_in, tag="ck")
                        nc.sync.dma_start(
                            out=ct[:], in_=ctxT[:, j * CPD * NPC : (j + 1) * CPD * NPC]
                        )
                        ctiles[j] = ct
                    if j < len(rtiles):
                        rt = rin.tile([128, RPD, M], dt_in, tag="rk")
                        nc.sync.dma_start(
                            out=rt[:], in_=rT[:, j * RPD * M : (j + 1) * RPD * M]
                        )
                        rtiles[j] = rt

                def ck_ap(k):
                    return ctiles[k // CPD][:, k % CPD : k % CPD + 1, :]

                def rk_ap(k):
                    return rtiles[k // RPD][:, k % RPD : k % RPD + 1, :]

                def ck2_ap(p):  # chunk pair (2p, 2p+1) as [128, 2, NPC]
                    k = 2 * p
                    return ctiles[k // CPD][:, k % CPD : k % CPD + 2, :]

                def rk2_ap(p):
                    k = 2 * p
                    return rtiles[k // RPD][:, k % RPD : k % RPD + 2, :]

                # squares: all ctx squares first (ctx lands before the
                # second rT transfer in the serialized DMA stream).
                # BK_SQ_FUSE squares F adjacent chunks per instruction: fewer
                # DVE fixed overheads (~105ns each) at coarser pipelining.
                # BK_ACT_SQ moves the last rT square ops to the ACT engine.
                FUSE = int(os.environ.get("BK_SQ_FUSE", "4"))
                FUSE_C = int(os.environ.get("BK_SQ_FUSE_C", str(FUSE)))
                assert KC % FUSE == 0 and KC % FUSE_C == 0
                nact = int(os.environ.get("BK_ACT_SQ", "1"))
                sqc_tiles, sqr_tiles = [None] * KC, [None] * KC
                sqc_big, sqr_big = [], []
                for j in range(KC // FUSE_C if not HOST_CSQ else 0):
                    sq_c = sqp.tile([128, FUSE_C, NPC], dt_in, tag="sqc")
                    # chunks j*FUSE_C..(j+1)*FUSE_C-1 are contiguous in the ctx tile
                    c0 = (j * FUSE_C) % CPD
                    src = ctiles[(j * FUSE_C) // CPD][:, c0 : c0 + FUSE_C, :]
                    if os.environ.get("BK_GP_SQC", "0") == "1":
                        # ctx squares on the otherwise-idle gpsimd/Pool engine
                        nc.gpsimd.tensor_mul(sq_c[:], src, src)
                    else:
                        nc.vector.tensor_mul(sq_c[:], src, src)
                    sqc_big.append(sq_c)
                    for f in range(FUSE_C):
                        sqc_tiles[j * FUSE_C + f] = sq_c[:, f : f + 1, :]
                for j in range(KC // FUSE if not HOST_RSQ else 0):
                    sq_r = sqp.tile([128, FUSE, M], dt_in, tag="sqr")
                    r0 = (j * FUSE) % RPD
                    src = rtiles[(j * FUSE) // RPD][:, r0 : r0 + FUSE, :]
                    if j >= KC // FUSE - nact:
                        # offload trailing rT squares to the otherwise-idle ACT
                        # engine (Square is in the natural_log_exp_and_others
                        # table set - no extra table load): the serial DVE
                        # square chain gates the final norm matmuls, which
                        # gate the whole epilogue
                        nc.scalar.square(sq_r[:], src)
                    else:
                        nc.vector.tensor_mul(sq_r[:], src, src)
                    sqr_big.append(sq_r)
                    for f in range(FUSE):
                        sqr_tiles[j * FUSE + f] = sq_r[:, f : f + 1, :]

                def sqc2_ap(p):  # square chunk pair as [128, 2, NPC]
                    k = 2 * p
                    return sqc_big[k // FUSE_C][:, k % FUSE_C : k % FUSE_C + 2, :]

                def sqr2_ap(p):
                    k = 2 * p
                    return sqr_big[k // FUSE][:, k % FUSE : k % FUSE + 2, :]

                # PE: interleave cross and square-norm matmuls in half-KC
                # groups so the LAST norm matmul (which gates the row copies
                # -> K=1 broadcasts -> Ln) retires as early as possible.
                if DR:
                    # DoubleRow: adjacent-chunk pairs, K=256 per matmul at
                    # 0.5 PE cycles/row - half the instructions, 2x the rate
                    NP2 = KC // 2
                    H2 = NP2 // 2
                    for g in range(2):
                        for p in range(g * H2, (g + 1) * H2):
                            nc.tensor.matmul(
                                P[:], ck2_ap(p), rk2_ap(p),
                                start=(p == 0), stop=False, perf_mode=DRM,
                            )
                        if g == 0 and os.environ.get("BK_LN_BIAS", "0") != "1":
                            nc.tensor.matmul(
                                P[:], mmean_row[:], ones_row[:],
                                start=False, stop=False,
                            )
                        for p in (range(g * H2, (g + 1) * H2) if not HOST_RSQ else ()):
                            nc.tensor.matmul(
                                rsq_ps[:], ones2[:], sqr2_ap(p),
                                start=(p == 0), stop=(p == NP2 - 1), perf_mode=DRM,
                            )
                        for p in (range(g * H2, (g + 1) * H2) if not HOST_CSQ else ()):
                            nc.tensor.matmul(
                                csq_ps[:], ones2[:], sqc2_ap(p),
                                start=(p == 0), stop=(p == NP2 - 1), perf_mode=DRM,
                            )
                else:
                    H = KC // 2
                    for g in range(2):
                        for k in range(g * H, (g + 1) * H):
                            # cross[n, m] += sum_d C[n, d] R[m, d]
                            nc.tensor.matmul(
                                P[:], mm(ck_ap(k)), mm(rk_ap(k)),
                                start=(k == 0), stop=False,
                            )
                        if g == 0 and os.environ.get("BK_LN_BIAS", "0") != "1":
                            # dep-free mean-restore term, off the critical tail
                            # (with BK_LN_BIAS the constant folds into the Ln
                            # bias instead - one PE matmul saved)
                            nc.tensor.matmul(
                                P[:], mmean_row[:], ones_row[:], start=False, stop=False
                            )
                        for k in (range(g * H, (g + 1) * H) if not HOST_RSQ else ()):
                            # r_sq[m] += sum_d R[m, d]^2  (all rsq before csq in
                            # each group: the rsq stop gates the copy -> K=1 -> Ln
                            # chain, so it must retire first)
                            nc.tensor.matmul(
                                rsq_ps[:], mm(ones_col[:]), mm(sqr_tiles[k]),
                                start=(k == 0), stop=(k == KC - 1),
                            )
                        for k in (range(g * H, (g + 1) * H) if not HOST_CSQ else ()):
                            # c_sq[n] += sum_d C[n, d]^2  (as a [1, NPC] row)
                            nc.tensor.matmul(
                                csq_ps[:], mm(ones_col[:]), mm(sqc_tiles[k]),
                                start=(k == 0), stop=(k == KC - 1),
                            )

                # Broadcast terms: K=1 matmuls add -0.5*(r_sq[m] + c_sq[n])
                # into P (PE does the partition-broadcast naturally).  The rows
                # are mean-centered by -D before the bf16 round so only small
                # residuals are quantized; the dep-free mean-restore matmul
                # already ran above.  The two PSUM->SBUF row copies run on
                # different engines (ACT + DVE) so they overlap; Copy is in
                # every activation-table set, so no extra table load.
                rsq_sb = epi.tile([1, M], row_dt)
                if HOST_RSQ:
                    rsq_sb = rsq_dma
                elif os.environ.get("BK_RSQ_GP", "0") == "1":
                    nc.gpsimd.tensor_scalar_add(rsq_sb[:], rsq_ps[:], -float(D))
                else:
                    nc.vector.tensor_scalar_add(rsq_sb[:], rsq_ps[:], -float(D))
                if HOST_CSQ:
                    csq_sb = csq_dma
                else:
                    csq_sb = epi.tile([1, NPC], row_dt)
                if not HOST_CSQ and os.environ.get("BK_CSQ_GP", "0") == "1":
                    # csq row copy on idle gpsimd: frees ~250ns of ACT
                    nc.gpsimd.tensor_scalar_add(csq_sb[:], csq_ps[:], -float(D))
                elif not HOST_CSQ:
                    nc.scalar.activation(csq_sb[:], csq_ps[:], AF.Copy, bias=-float(D))

                # csq-K1 first: its DVE copy retires before the ACT rsq copy,
                # so the stop=True matmul (gating the Ln) issues sooner
                nc.tensor.matmul(P[:], csq_sb[:], negh_row[:], start=False, stop=False)
                nc.tensor.matmul(P[:], neghalf[:], rsq_sb[:], start=False, stop=True)

                # dist = sqrt(-2*P) = exp(0.5*ln(-2*P)): Ln and Exp share one
                # activation-table set (natural_log_exp_and_others), so the
                # whole epilogue needs a single table load (hoisted to T~0 by
                # the ACT queue) instead of a 1.28us sqrt->exp table switch.
                # BK_EPI_BF16: run the whole Ln/Exp chain in bf16.  dist
                # bf16 (step 0.25 at dist~45) gives e rel err ~13%/element,
                # but softmax normalization cancels all but ~5e-4 on the
                # output - and 2-byte dtypes get the ACT/DVE 2x fast path.
                epi_dt = (
                    mybir.dt.bfloat16
                    if os.environ.get("BK_EPI_BF16", "0") == "1"
                    else f32
                )
                lg = epi.tile([NPC, M], epi_dt)
                if os.environ.get("BK_LN_BIAS", "0") == "1":
                    nc.scalar.activation(
                        lg[:], P[:], AF.Ln, scale=-2.0, bias=lnb_col[:]
                    )
                else:
                    nc.scalar.activation(lg[:], P[:], AF.Ln, scale=-2.0)
                dist = epi.tile([NPC, M], epi_dt)
                nc.scalar.activation(dist[:], lg[:], AF.Exp, scale=0.5)
                # e = exp(dist), s[n] = sum_m e[n, m]  (fused accumulation)
                e_dt = (
                    mybir.dt.bfloat16
                    if os.environ.get("BK_E_BF16", "1") == "1"
                    else f32
                )
                e = epi.tile([NPC, M], e_dt)
                s = epi.tile([NPC, 1], f32)
                nc.scalar.activation(e[:], dist[:], AF.Exp, accum_out=s[:])
                ns = epi.tile([NPC, 1], f32)
                nrcp = epi.tile([NPC, 1], f32)
                if os.environ.get("BK_NS_GP", "0") == "1":
                    nc.gpsimd.tensor_scalar_mul(ns[:], s[:], -1.0)
                    nc.gpsimd.reciprocal(nrcp[:], ns[:])
                else:
                    nc.vector.tensor_scalar_mul(ns[:], s[:], -1.0)
                    nc.vector.reciprocal(nrcp[:], ns[:])
                # out = e * (-1/s) + 1 = 1 - softmax
                osb = epi.tile([NPC, M], dt_out)
                if os.environ.get("BK_ACT_OUT", "0") == "1":
                    # on ACT: Copy(e * scale + bias) with per-partition scale
                    nc.scalar.activation(
                        osb[:], e[:], AF.Copy, scale=nrcp[:], bias=1.0
                    )
                else:
                    nc.vector.tensor_scalar(
                        osb[:], e[:], nrcp[:], 1.0,
                        mybir.AluOpType.mult, mybir.AluOpType.add,
                    )
                nc.sync.dma_start(out=out[:], in_=osb[:])

    nc.compile()
    return nc


def shard_inputs(context_embeddings: np.ndarray, result_embeddings: np.ndarray, mode: str = MODE):
    """Build per-core input maps in the contraction-major chunk layouts."""
    np_in = {
        "fp8": ml_dtypes.float8_e4m3,
        "bf16": ml_dtypes.bfloat16,
    }.get(mode, np.float32)
    C = np.asarray(context_embeddings, dtype=np.float32)
    R = np.asarray(result_embeddings, dtype=np.float32)

    # rT[p, k*M + m] = R[m, k*128 + p]
    rT = np.ascontiguousarray(
        R.T.reshape(KC, 128, M).transpose(1, 0, 2).reshape(128, KC * M)
    ).astype(np_in)
    host_rsq = os.environ.get("BK_HOST_RSQ", "1") == "1"
    host_csq = os.environ.get("BK_HOST_CSQ", "1") == "1"
    if host_rsq:
        # database norms of the QUANTIZED vectors (consistent with the
        # on-device cross products), mean-centered for the bf16 round
        Rq = R.astype(np_in).astype(np.float32)
        rsqrow = (
            ((Rq**2).sum(axis=1) - float(D))
            .reshape(1, M)
            .astype(ml_dtypes.bfloat16)
        )

    in_maps = []
    for i in range(NCORES):
        Ci = C[i * NPC : (i + 1) * NPC]  # [NPC, D]
        ctxT = np.ascontiguousarray(
            Ci.T.reshape(KC, 128, NPC).transpose(1, 0, 2).reshape(128, KC * NPC)
        ).astype(np_in)
        m = {"ctxT": ctxT, "rT": rT}
        if host_rsq:
            m["rsqrow"] = rsqrow
        if host_csq:
            Cq = Ci.astype(np_in).astype(np.float32)
            m["csqrow"] = (
                ((Cq**2).sum(axis=1) - float(D))
                .reshape(1, NPC)
                .astype(ml_dtypes.bfloat16)
            )
        in_maps.append(m)
    return in_maps


def kernel(**inputs) -> np.ndarray:
    in_maps = shard_inputs(
        inputs["context_embeddings"], inputs["result_embeddings"], MODE
    )
    nc = build_nc(MODE)
    res = run_bass_kernel_spmd(nc, in_maps, core_ids=list(range(NCORES)))
    return np.concatenate(
        [res.results[i]["out"].astype(np.float32) for i in range(NCORES)], axis=0
    )



# revision 31
# speedup vs baseline: 2.8869x; 2.8869x over previous
"""Trainium2 Bass kernel: pairwise L2 distance + softmax classifier head.

reference math (per row n of context, all m result rows):
    sq[n, m] = ||c_n||^2 - 2 c_n.r_m + ||r_m||^2
    out[n, m] = 1 - softmax_m(sqrt(sq[n, m]))

Strategy
- Data-parallel over the context axis: 8 cores x 128 context rows each;
  result_embeddings replicated to every core.
- Host pre-transposes both operands into contraction-major ("[d, .]")
  chunk layouts so the tensor engine needs no on-chip transposes.
- Per core: 8 accumulating matmuls give cross = C_i @ R^T in PSUM
  (interleaved in half-K groups with the ones-weight square-norm
  reductions so the epilogue-gating matmuls retire early); K=1 matmuls
  broadcast -0.5*(r_sq[m] + c_sq[n]) into the same PSUM (rows
  mean-centered by -D so bf16 rounds only small residuals); the scalar
  engine computes dist = exp(0.5*ln(-2P)) and e = exp(dist) with fused
  row-sum accumulation, and DVE finishes with 1 - e/s.  sqrt-as-
  exp(ln/2) keeps every activation in ONE table set
  (natural_log_exp_and_others, forced via the insert_act_table_loads
  patch) - each avoided table switch is 1.28us.  No softmax
  max-subtraction is needed: distances <= ~51, far below f32 overflow.
- Built on bacc.Bacc + compile(): the ISA Events field fits exactly one
  semaphore wait + one update per instruction; bacc's passes lower the
  Tile-generated multi-wait sync into encodable event-semaphore chains.

Steady-state tuning (measured via in-NEFF repeat slope on HW, which
cancels the ~3.7ms/dispatch axon PJRT overhead and the ~69ms tunnel
round-trip; see test.py):
- BK_SQ_FUSE=4 (default): square 4 contraction chunks per DVE
  instruction instead of 1 - each DVE instruction carries ~105ns of
  fixed overhead (58-cycle SBUF access + 45ns SEQ), so 16 small squares
  waste ~1.4us/iteration; groups of 4 keep enough pipelining against
  the PE norm-matmul consumers while cutting 12 of those overheads.
- BK_ACT_SQ=1 (default): the last rT square group runs on the ACT
  engine (Square is in the natural_log_exp_and_others table set - no
  table switch), rebalancing the DVE-critical square chain.
- BK_OUT_BF16=1 (default): bf16 output DMA (64KB not 128KB per core);
  output values are ~1 so bf16 keeps rel err ~2.4e-3, far inside the
  2e-2 gate. kernel() casts back to float32 on host.
HW steady-state: ~2.6us/iteration per core (from 3.5us before these
three changes; TimelineSim models 2.9us). Engines are near-balanced:
DVE ~2.3us, PE ~2.4us, ACT ~2.3us busy per iteration.
Rejected by measurement/sim: fp8 inputs (no PE-rate or total-time win
in the cost model, DMA is not the binding resource), fully-fused
single-op squares (serializes the norm-matmul tail), final combine on
ACT (ACT is critical in the epilogue chain), pre-reducing square
chunks on DVE (DVE is critical), 1 or 4 rT DMAs (2 is best).
"""

import os

import ml_dtypes
import numpy as np

import concourse.bass as bass
import concourse.mybir as mybir
import concourse.tile as tile
from concourse import bacc
from concourse.bass_utils import run_bass_kernel_spmd

N, D, M = 1024, 1024, 256
NCORES = 8
NPC = N // NCORES  # context rows per core (output partition dim)
KC = D // 128  # contraction chunks of 128

# "bf16": inputs cast to bf16 on host (half the DMA bytes, ~5e-4 rel err)
# "fp8" : inputs fp8 e4m3; cross+norm matmuls in MatmulPerfMode.DoubleRow
#         (0.5 PE cycles/row, K=256/instruction, halves PE time and input
#         DMA; ~1e-2 rel err). REJECTED: fp8 is a 1-byte dtype, so the
#         DVE/ACT elementwise 2x fast path (2-byte only) is lost and the
#         square ops double in cost - sim: 3046ns/body (best placement,
#         ctx squares on gpsimd) vs 2912 for bf16. ALSO fails walrus
#         lowering in this env (CallFunctionObjArgs error in lower_act/
#         lower_dve). Kept for reference only.
# "f32r": f32 inputs, matmuls in float32r (full-rate PE, ~tf32 products)
# "f32" : f32 inputs, plain fp32 matmuls (1/4-rate PE, ~2e-6 rel err)
MODE = os.environ.get("BASS_KERNEL_MODE", "bf16")


def build_nc(mode: str = MODE) -> bass.Bass:
    AF = mybir.ActivationFunctionType
    f32 = mybir.dt.float32
    dt_in = {
        "fp8": mybir.dt.float8e4,
        "bf16": mybir.dt.bfloat16,
        "f32r": mybir.dt.float32r,
        "f32": f32,
    }[mode]
    DR = mode == "fp8"  # DoubleRow: pair adjacent K-chunks per matmul
    DRM = mybir.MatmulPerfMode.DoubleRow

    def mm(ap):
        return ap

    # bacc (not raw Bass): its compile() pass lowers multi-wait sync_info
    # into event-semaphore sequences the ISA can actually encode (the
    # Events field fits exactly one wait + one update per instruction).
    nc = bacc.Bacc("TRN2", target_bir_lowering=False, debug=False, num_devices=NCORES)

    # The act-table chooser is first-match greedy; hide the pure-Ln set so
    # the Ln lands in natural_log_exp_and_others and the epilogue's Ln/Exp
    # chain needs exactly ONE activation-table load (each switch is 1.28us).
    def _act_loads(self=nc):
        import bass_rust as _bass_rust
        from concourse.hw_specs import get_activation_tables
        has_act = any(
            isinstance(i, mybir.InstActivation)
            for b in self.main_func.blocks
            for i in b.instructions
        )
        if not has_act:
            return
        tables = [
            (name, s if name == "natural_log_exp_and_others" else set())
            for name, s in get_activation_tables(self.m.arch).items()
        ]
        _bass_rust.insert_act_table_loads(self, tables)

    nc.insert_act_table_loads = _act_loads
    # ctxT[p, k*NPC + n] = C[core*NPC + n, k*128 + p]
    ctxT = nc.dram_tensor("ctxT", [128, KC * NPC], dt_in, kind="ExternalInput")
    # rT[p, k*M + m] = R[m, k*128 + p]
    # BK_RT_FP8: store the (replicated, largest) database operand in fp8 -
    # matmul operand dtypes are independent unless f32; with host norms
    # there are no fp8 elementwise ops left to pay the 1-byte DVE penalty
    dt_rt = (
        mybir.dt.float8e4
        if os.environ.get("BK_RT_FP8", "0") == "1"
        else dt_in
    )
    rT = nc.dram_tensor("rT", [128, KC * M], dt_rt, kind="ExternalInput")
    HOST_RSQ = os.environ.get("BK_HOST_RSQ", "1") == "1"
    HOST_CSQ = os.environ.get("BK_HOST_CSQ", "1") == "1"
    if HOST_CSQ:
        # ||c_n||^2 - D for this core's query rows, precomputed on host in
        # f32 (more accurate than the device bf16 square path) - [1, NPC]
        csqrow = nc.dram_tensor(
            "csqrow", [1, NPC], mybir.dt.bfloat16, kind="ExternalInput"
        )
    if HOST_RSQ:
        # ||r_m||^2 - D, precomputed with the (fixed) result-embedding
        # database on host - standard retrieval practice - as a bf16 row
        rsqrow = nc.dram_tensor(
            "rsqrow", [1, M], mybir.dt.bfloat16, kind="ExternalInput"
        )
    dt_out = mybir.dt.bfloat16 if os.environ.get("BK_OUT_BF16", "1") == "1" else f32
    out = nc.dram_tensor("out", [NPC, M], dt_out, kind="ExternalOutput")

    with tile.TileContext(nc) as tc:
        with (
            tc.tile_pool(name="cin", bufs=KC // 2) as cin,
            tc.tile_pool(name="rin", bufs=KC) as rin,
            tc.tile_pool(name="sq", bufs=KC) as sqp,
            tc.tile_pool(name="consts", bufs=1) as consts,
            tc.tile_pool(
                name="epi", bufs=int(os.environ.get("BK_EPI_BUFS", "1"))
            ) as epi,
            tc.tile_pool(
                name="psum", bufs=int(os.environ.get("BK_PSUM_BUFS", "1")),
                space="PSUM",
            ) as psp,
        ):
            row_dt = mybir.dt.bfloat16 if mode in ("bf16", "fp8") else f32
            ones_col = consts.tile([128, 1], dt_in)
            nc.vector.memset(ones_col, 1.0)
            if DR:
                # [128, 2, 1] ones: DoubleRow stationary operand for the
                # ones-weight norm reductions (2 K-chunks per matmul)
                ones2 = consts.tile([128, 2, 1], dt_in)
                nc.vector.memset(ones2, 1.0)
            neghalf = consts.tile([1, NPC], row_dt)
            nc.vector.memset(neghalf, -0.5)
            negh_row = consts.tile([1, M], row_dt)
            nc.vector.memset(negh_row, -0.5)
            mmean_row = consts.tile([1, NPC], row_dt)
            nc.vector.memset(mmean_row, -float(D))
            ones_row = consts.tile([1, M], row_dt)
            nc.vector.memset(ones_row, 1.0)
            if os.environ.get("BK_LN_BIAS", "0") == "1":
                # per-partition +2D bias column for the Ln (large scalar
                # biases need a const AP, immediates won't lower)
                lnb_col = consts.tile([128, 1], f32)
                nc.vector.memset(lnb_col, 2.0 * float(D))

            # HAM warmup: the PE clock-gate defaults to 1.2GHz and needs
            # ~3.4us of sustained activity to unthrottle to 2.4GHz.  The real
            # matmuls only start after the ~4us DMA wait, so without this
            # they all run cold.  Burn dummy matmuls on a const tile during
            # the DMA wait; results are discarded.
            nwarm = int(os.environ.get("BK_WARM_MM", "16"))
            if nwarm:
                warm_in = consts.tile([128, NPC], dt_in)
                nc.vector.memset(warm_in, 1.0)
                warm_ps = psp.tile([1, NPC], f32, tag="warm")
                for _ in range(nwarm):
                    nc.tensor.matmul(
                        warm_ps[:], ones_col[:], warm_in[:], start=True, stop=True
                    )

            if HOST_RSQ:
                # loop-invariant 512B row: own HWDGE queue, overlaps the big
                # input DMAs in the real kernel (adds ~0 to its makespan)
                rsq_dma = consts.tile([1, M], row_dt)
                nc.sync.dma_start(out=rsq_dma[:], in_=rsqrow[:, :])
            if HOST_CSQ:
                csq_dma = consts.tile([1, NPC], row_dt)
                nc.sync.dma_start(out=csq_dma[:], in_=csqrow[:, :])

            # BK_REPEAT replicates the body inside one NEFF (benchmarking:
            # separates per-iteration throughput from fixed overhead)
            for _rep in range(int(os.environ.get("BK_REPEAT", "1"))):
                P = psp.tile([NPC, M], f32, tag="P")
                rsq_ps = psp.tile([1, M], f32, tag="rsq")
                csq_ps = psp.tile([1, NPC], f32, tag="csq")

                # <=8 DMAs total so each rides its own HWDGE completion lane (a
                # reused lane forces a serializing queue wait on the out-DMA,
                # blowing the 1-wait-per-instruction sync budget).  Per-DMA fixed
                # costs (HWDGE issue + completion sem) are ~1.5us, so keep the
                # count low.
                CPD = KC // int(os.environ.get("BK_CTX_DMAS", "1"))
                RPD = KC // int(os.environ.get("BK_RT_DMAS", "1"))
                ctiles = [None] * (KC // CPD)
                rtiles = [None] * (KC // RPD)
                # interleave issue order (ctx0, rT0, ctx1, rT1, ...) so the
                # first cross-matmul's inputs finish the serialized transfer
                # stream as early as possible
                for j in range(max(len(ctiles), len(rtiles))):
                    if j < len(ctiles):
                        # 3D [128, CPD, NPC]: chunk index explicit so DR mode
                        # can slice adjacent-chunk pairs as [128, 2, NPC] APs
                        ct = cin.tile([128, CPD, NPC], dt_in, tag="ck")
                        nc.sync.dma_start(
                            out=ct[:], in_=ctxT[:, j * CPD * NPC : (j + 1) * CPD * NPC]
                        )
                        ctiles[j] = ct
                    if j < len(rtiles):
                        rt = rin.tile([128, RPD, M], dt_rt, tag="rk")
                        nc.sync.dma_start(
                            out=rt[:], in_=rT[:, j * RPD * M : (j + 1) * RPD * M]
                        )
                        rtiles[j] = rt

                def ck_ap(k):
                    return ctiles[k // CPD][:, k % CPD : k % CPD + 1, :]

                def rk_ap(k):
                    return rtiles[k // RPD][:, k % RPD : k % RPD + 1, :]

                def ck2_ap(p):  # chunk pair (2p, 2p+1) as [128, 2, NPC]
                    k = 2 * p
                    return ctiles[k // CPD][:, k % CPD : k % CPD + 2, :]

                def rk2_ap(p):
                    k = 2 * p
                    return rtiles[k // RPD][:, k % RPD : k % RPD + 2, :]

                # squares: all ctx squares first (ctx lands before the
                # second rT transfer in the serialized DMA stream).
                # BK_SQ_FUSE squares F adjacent chunks per instruction: fewer
                # DVE fixed overheads (~105ns each) at coarser pipelining.
                # BK_ACT_SQ moves the last rT square ops to the ACT engine.
                FUSE = int(os.environ.get("BK_SQ_FUSE", "4"))
                FUSE_C = int(os.environ.get("BK_SQ_FUSE_C", str(FUSE)))
                assert KC % FUSE == 0 and KC % FUSE_C == 0
                nact = int(os.environ.get("BK_ACT_SQ", "1"))
                sqc_tiles, sqr_tiles = [None] * KC, [None] * KC
                sqc_big, sqr_big = [], []
                for j in range(KC // FUSE_C if not HOST_CSQ else 0):
                    sq_c = sqp.tile([128, FUSE_C, NPC], dt_in, tag="sqc")
                    # chunks j*FUSE_C..(j+1)*FUSE_C-1 are contiguous in the ctx tile
                    c0 = (j * FUSE_C) % CPD
                    src = ctiles[(j * FUSE_C) // CPD][:, c0 : c0 + FUSE_C, :]
                    if os.environ.get("BK_GP_SQC", "0") == "1":
                        # ctx squares on the otherwise-idle gpsimd/Pool engine
                        nc.gpsimd.tensor_mul(sq_c[:], src, src)
                    else:
                        nc.vector.tensor_mul(sq_c[:], src, src)
                    sqc_big.append(sq_c)
                    for f in range(FUSE_C):
                        sqc_tiles[j * FUSE_C + f] = sq_c[:, f : f + 1, :]
                for j in range(KC // FUSE if not HOST_RSQ else 0):
                    sq_r = sqp.tile([128, FUSE, M], dt_rt, tag="sqr")
                    r0 = (j * FUSE) % RPD
                    src = rtiles[(j * FUSE) // RPD][:, r0 : r0 + FUSE, :]
                    if j >= KC // FUSE - nact:
                        # offload trailing rT squares to the otherwise-idle ACT
                        # engine (Square is in the natural_log_exp_and_others
                        # table set - no extra table load): the serial DVE
                        # square chain gates the final norm matmuls, which
                        # gate the whole epilogue
                        nc.scalar.square(sq_r[:], src)
                    else:
                        nc.vector.tensor_mul(sq_r[:], src, src)
                    sqr_big.append(sq_r)
                    for f in range(FUSE):
                        sqr_tiles[j * FUSE + f] = sq_r[:, f : f + 1, :]

                def sqc2_ap(p):  # square chunk pair as [128, 2, NPC]
                    k = 2 * p
                    return sqc_big[k // FUSE_C][:, k % FUSE_C : k % FUSE_C + 2, :]

                def sqr2_ap(p):
                    k = 2 * p
                    return sqr_big[k // FUSE][:, k % FUSE : k % FUSE + 2, :]

                # PE: interleave cross and square-norm matmuls in half-KC
                # groups so the LAST norm matmul (which gates the row copies
                # -> K=1 broadcasts -> Ln) retires as early as possible.
                if DR:
                    # DoubleRow: adjacent-chunk pairs, K=256 per matmul at
                    # 0.5 PE cycles/row - half the instructions, 2x the rate
                    NP2 = KC // 2
                    H2 = NP2 // 2
                    for g in range(2):
                        for p in range(g * H2, (g + 1) * H2):
                            nc.tensor.matmul(
                                P[:], ck2_ap(p), rk2_ap(p),
                                start=(p == 0), stop=False, perf_mode=DRM,
                            )
                        if g == 0 and os.environ.get("BK_LN_BIAS", "0") != "1":
                            nc.tensor.matmul(
                                P[:], mmean_row[:], ones_row[:],
                                start=False, stop=False,
                            )
                        for p in (range(g * H2, (g + 1) * H2) if not HOST_RSQ else ()):
                            nc.tensor.matmul(
                                rsq_ps[:], ones2[:], sqr2_ap(p),
                                start=(p == 0), stop=(p == NP2 - 1), perf_mode=DRM,
                            )
                        for p in (range(g * H2, (g + 1) * H2) if not HOST_CSQ else ()):
                            nc.tensor.matmul(
                                csq_ps[:], ones2[:], sqc2_ap(p),
                                start=(p == 0), stop=(p == NP2 - 1), perf_mode=DRM,
                            )
                else:
                    H = KC // 2
                    for g in range(2):
                        for k in range(g * H, (g + 1) * H):
                            # cross[n, m] += sum_d C[n, d] R[m, d]
                            nc.tensor.matmul(
                                P[:], mm(ck_ap(k)), mm(rk_ap(k)),
                                start=(k == 0), stop=False,
                            )
                        if g == 0 and os.environ.get("BK_LN_BIAS", "0") != "1":
                            # dep-free mean-restore term, off the critical tail
                            # (with BK_LN_BIAS the constant folds into the Ln
                            # bias instead - one PE matmul saved)
                            nc.tensor.matmul(
                                P[:], mmean_row[:], ones_row[:], start=False, stop=False
                            )
                        for k in (range(g * H, (g + 1) * H) if not HOST_RSQ else ()):
                            # r_sq[m] += sum_d R[m, d]^2  (all rsq before csq in
                            # each group: the rsq stop gates the copy -> K=1 -> Ln
                            # chain, so it must retire first)
                            nc.tensor.matmul(
                                rsq_ps[:], mm(ones_col[:]), mm(sqr_tiles[k]),
                                start=(k == 0), stop=(k == KC - 1),
                            )
                        for k in (range(g * H, (g + 1) * H) if not HOST_CSQ else ()):
                            # c_sq[n] += sum_d C[n, d]^2  (as a [1, NPC] row)
                            nc.tensor.matmul(
                                csq_ps[:], mm(ones_col[:]), mm(sqc_tiles[k]),
                                start=(k == 0), stop=(k == KC - 1),
                            )

                # Broadcast terms: K=1 matmuls add -0.5*(r_sq[m] + c_sq[n])
                # into P (PE does the partition-broadcast naturally).  The rows
                # are mean-centered by -D before the bf16 round so only small
                # residuals are quantized; the dep-free mean-restore matmul
                # already ran above.  The two PSUM->SBUF row copies run on
                # different engines (ACT + DVE) so they overlap; Copy is in
                # every activation-table set, so no extra table load.
                rsq_sb = epi.tile([1, M], row_dt)
                if HOST_RSQ:
                    rsq_sb = rsq_dma
                elif os.environ.get("BK_RSQ_GP", "0") == "1":
                    nc.gpsimd.tensor_scalar_add(rsq_sb[:], rsq_ps[:], -float(D))
                else:
                    nc.vector.tensor_scalar_add(rsq_sb[:], rsq_ps[:], -float(D))
                if HOST_CSQ:
                    csq_sb = csq_dma
                else:
                    csq_sb = epi.tile([1, NPC], row_dt)
                if not HOST_CSQ and os.environ.get("BK_CSQ_GP", "0") == "1":
                    # csq row copy on idle gpsimd: frees ~250ns of ACT
                    nc.gpsimd.tensor_scalar_add(csq_sb[:], csq_ps[:], -float(D))
                elif not HOST_CSQ:
                    nc.scalar.activation(csq_sb[:], csq_ps[:], AF.Copy, bias=-float(D))

                # csq-K1 first: its DVE copy retires before the ACT rsq copy,
                # so the stop=True matmul (gating the Ln) issues sooner
                nc.tensor.matmul(P[:], csq_sb[:], negh_row[:], start=False, stop=False)
                nc.tensor.matmul(P[:], neghalf[:], rsq_sb[:], start=False, stop=True)

                # dist = sqrt(-2*P) = exp(0.5*ln(-2*P)): Ln and Exp share one
                # activation-table set (natural_log_exp_and_others), so the
                # whole epilogue needs a single table load (hoisted to T~0 by
                # the ACT queue) instead of a 1.28us sqrt->exp table switch.
                # BK_EPI_BF16: run the whole Ln/Exp chain in bf16.  dist
                # bf16 (step 0.25 at dist~45) gives e rel err ~13%/element,
                # but softmax normalization cancels all but ~5e-4 on the
                # output - and 2-byte dtypes get the ACT/DVE 2x fast path.
                epi_dt = (
                    mybir.dt.bfloat16
                    if os.environ.get("BK_EPI_BF16", "0") == "1"
                    else f32
                )
                lg = epi.tile([NPC, M], epi_dt)
                if os.environ.get("BK_LN_BIAS", "0") == "1":
                    nc.scalar.activation(
                        lg[:], P[:], AF.Ln, scale=-2.0, bias=lnb_col[:]
                    )
                else:
                    nc.scalar.activation(lg[:], P[:], AF.Ln, scale=-2.0)
                dist = epi.tile([NPC, M], epi_dt)
                nc.scalar.activation(dist[:], lg[:], AF.Exp, scale=0.5)
                # e = exp(dist), s[n] = sum_m e[n, m]  (fused accumulation)
                e_dt = (
                    mybir.dt.bfloat16
                    if os.environ.get("BK_E_BF16", "1") == "1"
                    else f32
                )
                e = epi.tile([NPC, M], e_dt)
                s = epi.tile([NPC, 1], f32)
                nc.scalar.activation(e[:], dist[:], AF.Exp, accum_out=s[:])
                ns = epi.tile([NPC, 1], f32)
                nrcp = epi.tile([NPC, 1], f32)
                if os.environ.get("BK_NS_GP", "0") == "1":
                    nc.gpsimd.tensor_scalar_mul(ns[:], s[:], -1.0)
                    nc.gpsimd.reciprocal(nrcp[:], ns[:])
                else:
                    nc.vector.tensor_scalar_mul(ns[:], s[:], -1.0)
                    nc.vector.reciprocal(nrcp[:], ns[:])
                # out = e * (-1/s) + 1 = 1 - softmax
                osb = epi.tile([NPC, M], dt_out)
                if os.environ.get("BK_ACT_OUT", "0") == "1":
                    # on ACT: Copy(e * scale + bias) with per-partition scale
                    nc.scalar.activation(
                        osb[:], e[:], AF.Copy, scale=nrcp[:], bias=1.0
                    )
                else:
                    nc.vector.tensor_scalar(
                        osb[:], e[:], nrcp[:], 1.0,
                        mybir.AluOpType.mult, mybir.AluOpType.add,
                    )
                nc.sync.dma_start(out=out[:], in_=osb[:])

    nc.compile()
    return nc


def shard_inputs(context_embeddings: np.ndarray, result_embeddings: np.ndarray, mode: str = MODE):
    """Build per-core input maps in the contraction-major chunk layouts."""
    np_in = {
        "fp8": ml_dtypes.float8_e4m3,
        "bf16": ml_dtypes.bfloat16,
    }.get(mode, np.float32)
    C = np.asarray(context_embeddings, dtype=np.float32)
    R = np.asarray(result_embeddings, dtype=np.float32)

    # rT[p, k*M + m] = R[m, k*128 + p]
    np_rt = (
        ml_dtypes.float8_e4m3
        if os.environ.get("BK_RT_FP8", "0") == "1"
        else np_in
    )
    rT = np.ascontiguousarray(
        R.T.reshape(KC, 128, M).transpose(1, 0, 2).reshape(128, KC * M)
    ).astype(np_rt)
    host_rsq = os.environ.get("BK_HOST_RSQ", "1") == "1"
    host_csq = os.environ.get("BK_HOST_CSQ", "1") == "1"
    if host_rsq:
        # database norms of the QUANTIZED vectors (consistent with the
        # on-device cross products), mean-centered for the bf16 round
        Rq = R.astype(np_rt).astype(np.float32)
        rsqrow = (
            ((Rq**2).sum(axis=1) - float(D))
            .reshape(1, M)
            .astype(ml_dtypes.bfloat16)
        )

    in_maps = []
    for i in range(NCORES):
        Ci = C[i * NPC : (i + 1) * NPC]  # [NPC, D]
        ctxT = np.ascontiguousarray(
            Ci.T.reshape(KC, 128, NPC).transpose(1, 0, 2).reshape(128, KC * NPC)
        ).astype(np_in)
        m = {"ctxT": ctxT, "rT": rT}
        if host_rsq:
            m["rsqrow"] = rsqrow
        if host_csq:
            Cq = Ci.astype(np_in).astype(np.float32)
            m["csqrow"] = (
                ((Cq**2).sum(axis=1) - float(D))
                .reshape(1, NPC)
                .astype(ml_dtypes.bfloat16)
            )
        in_maps.append(m)
    return in_maps


def kernel(**inputs) -> np.ndarray:
    in_maps = shard_inputs(
        inputs["context_embeddings"], inputs["result_embeddings"], MODE
    )
    nc = build_nc(MODE)
    res = run_bass_kernel_spmd(nc, in_maps, core_ids=list(range(NCORES)))
    return np.concatenate(
        [res.results[i]["out"].astype(np.float32) for i in range(NCORES)], axis=0
    )

